# revision 1
# baseline (speedup 1.0000x reference)
"""LowRankAttention Trainium2 kernel (Bass/Tile), data-parallel over 8 NeuronCores.

Math per batch b (one batch per core):
    Q = q @ Wq^T, K = k @ Wk^T, V = v @ Wv^T          (rank projections, R=256)
    A = softmax(Q K^T / sqrt(R))                       (softmax over keys j)
    out = (A @ V) @ Wo^T

Device-side layout strategy (PE contracts over the partition dim):
  - Host feeds each core qT/kT/vT = q[b].T etc ([D, S], zero-FLOP layout prep),
    so projections run directly: stationary = W^T tiles / vT tiles.
  - Q^T, K^T kept as [R, S]; V as [S, R]; A^T tiles [j, i] come straight from
    lhsT=K^T, rhs=Q^T.  exp() on ScalarE (no max-subtraction needed: |A|<~7).
  - Unnormalized E^T tiles feed EV (lhsT=V tile) -> AV^T, and a ones-column
    matmul accumulates row sums.  Normalization (1/sum) is folded into the
    final output-projection copy (per-partition scale on ScalarE).
  - All matmuls use float32r (TF32-like, full PE speed, ~1.5e-4 rel error).
  - Load order k -> q -> v with 1 MiB DMAs; attention chunk 0 starts once
    K^T and Q^T[chunk0] exist while v still streams in.
"""

import numpy as np

import concourse.bacc as bacc
import concourse.mybir as mybir
import concourse.tile as tile
from concourse import bass_utils

F32 = mybir.dt.float32
F32R = mybir.dt.float32r
AF = mybir.ActivationFunctionType

DIM, RANK, B, S = 1024, 256, 8, 2048
P = 128
NC = 512                      # moving-operand / psum free chunk
DT = DIM // P                 # 8  d-tiles
RT = RANK // P                # 2  r-tiles
SC = S // NC                  # 4  s-chunks (i-chunks)
JT = S // P                   # 16 j-tiles
SCALE = 1.0 / np.sqrt(np.float32(RANK))


def build_program(reps: int = 1):
    """Build + compile the per-core Bass program. reps>1 wraps the whole body
    in a For_i loop (used only for wall-clock timing)."""
    nc = bacc.Bacc("TRN2", target_bir_lowering=False, debug=False)

    qT = nc.dram_tensor("qT", [DIM, S], F32, kind="ExternalInput")
    kT = nc.dram_tensor("kT", [DIM, S], F32, kind="ExternalInput")
    vT = nc.dram_tensor("vT", [DIM, S], F32, kind="ExternalInput")
    wqT = nc.dram_tensor("wqT", [DIM, RANK], F32, kind="ExternalInput")
    wkT = nc.dram_tensor("wkT", [DIM, RANK], F32, kind="ExternalInput")
    wvT = nc.dram_tensor("wvT", [DIM, RANK], F32, kind="ExternalInput")
    woT = nc.dram_tensor("woT", [RANK, DIM], F32, kind="ExternalInput")
    out = nc.dram_tensor("out", [S, DIM], F32, kind="ExternalOutput")

    with tile.TileContext(nc) as tc:
        with tc.tile_pool(name="w", bufs=1) as wpool, \
             tc.tile_pool(name="inp", bufs=10) as inpool, \
             tc.tile_pool(name="inq", bufs=10) as qpool, \
             tc.tile_pool(name="per", bufs=1) as perpool, \
             tc.tile_pool(name="et", bufs=6) as etpool, \
             tc.tile_pool(name="av", bufs=4) as avpool, \
             tc.tile_pool(name="o", bufs=3) as opool, \
             tc.tile_pool(name="sm", bufs=4) as smpool, \
             tc.tile_pool(name="ps", bufs=3, space="PSUM") as pspool, \
             tc.tile_pool(name="pso", bufs=2, space="PSUM") as psopool, \
             tc.tile_pool(name="psav", bufs=2, space="PSUM") as psavpool, \
             tc.tile_pool(name="pssum", bufs=1, space="PSUM") as pssumpool, \
             tc.tile_pool(name="dr", bufs=4, space="DRAM") as drpool:

            def body(_i=None):
                # ---- weights ----
                wq_t = wpool.tile([P, DT, RANK], F32R, tag="wq", name="wq_t")
                wk_t = wpool.tile([P, DT, RANK], F32R, tag="wk", name="wk_t")
                wv_t = wpool.tile([P, DT, RANK], F32R, tag="wv", name="wv_t")
                wo_t = wpool.tile([P, RT, DIM], F32R, tag="wo", name="wo_t")
                nc.sync.dma_start(wk_t[:], wkT.ap().rearrange("(dt p) r -> p dt r", p=P).bitcast(F32R))
                nc.sync.dma_start(wq_t[:], wqT.ap().rearrange("(dt p) r -> p dt r", p=P).bitcast(F32R))
                nc.sync.dma_start(wv_t[:], wvT.ap().rearrange("(dt p) r -> p dt r", p=P).bitcast(F32R))
                nc.sync.dma_start(wo_t[:], woT.ap().rearrange("(rt p) d -> p rt d", p=P).bitcast(F32R))
                ones_f = wpool.tile([P, 1], F32, tag="onesf", name="ones_f")
                nc.vector.memset(ones_f[:], 1.0)
                ones = wpool.tile([P, 1], F32R, tag="ones", name="ones")
                nc.vector.tensor_copy(ones[:], ones_f[:])

                # ---- projections ----
                QT_t = perpool.tile([P, RT, S], F32R, tag="QT", name="QT_t")   # [r_p, rt, i]
                KT_t = perpool.tile([P, RT, S], F32R, tag="KT", name="KT_t")   # [r_p, rt, j]
                V_t = perpool.tile([P, JT, RANK], F32R, tag="V", name="V_t")   # [j_p, jt, r]

                H = S // 2

                def load_halves(src):
                    tiles = {}
                    for h in range(2):
                        for dt in range(DT):
                            t = inpool.tile([P, H], F32R, tag="inH", name=f"inH_{dt}_{h}")
                            nc.sync.dma_start(
                                t[:], src.ap()[dt * P:(dt + 1) * P, h * H:(h + 1) * H].bitcast(F32R))
                            tiles[(dt, h)] = t
                    return tiles

                # k fully, then v fully (j-side data first); q streams per chunk.
                ktiles = load_halves(kT)
                vtiles = load_halves(vT)
                qtiles = {}
                for ic in range(SC):
                    for dt in range(DT):
                        t = qpool.tile([P, NC], F32R, tag="inQ", name=f"inQ_{dt}")
                        nc.sync.dma_start(
                            t[:], qT.ap()[dt * P:(dt + 1) * P, ic * NC:(ic + 1) * NC].bitcast(F32R))
                        qtiles[(dt, ic)] = t

                # K^T projection (j-side), per s-chunk
                for sc in range(SC):
                    h, o = sc // 2, (sc % 2) * NC
                    for rt in range(RT):
                        ps = pspool.tile([P, NC], F32, tag="ps", name="ps_projk")
                        for dt in range(DT):
                            nc.tensor.matmul(ps[:], wk_t[:, dt, rt * P:(rt + 1) * P],
                                             ktiles[(dt, h)][:, o:o + NC],
                                             start=(dt == 0), stop=(dt == DT - 1))
                        nc.scalar.copy(KT_t[:, rt, sc * NC:(sc + 1) * NC], ps[:])

                # V projection
                for jt in range(JT):
                    h, o = jt // 8, (jt % 8) * P
                    ps = psopool.tile([P, NC], F32, tag="pso", name="ps_v")
                    psv = ps[:, :RANK]
                    for dt in range(DT):
                        nc.tensor.matmul(psv, vtiles[(dt, h)][:, o:o + P], wv_t[:, dt, :],
                                         start=(dt == 0), stop=(dt == DT - 1))
                    nc.scalar.copy(V_t[:, jt, :], psv)

                # ---- per-chunk: Q^T proj + attention (lag-2 A^T -> EV pipeline) ----
                LAG = 2
                for ic in range(SC):
                    isl = slice(ic * NC, (ic + 1) * NC)
                    for rt in range(RT):
                        ps = pspool.tile([P, NC], F32, tag="ps", name="ps_projq")
                        for dt in range(DT):
                            nc.tensor.matmul(ps[:], wq_t[:, dt, rt * P:(rt + 1) * P],
                                             qtiles[(dt, ic)][:],
                                             start=(dt == 0), stop=(dt == DT - 1))
                        nc.scalar.copy(QT_t[:, rt, isl], ps[:])

                    av_ps = [psavpool.tile([P, NC], F32, tag="av", name=f"av_{rt}") for rt in range(RT)]
                    sum_ps = pssumpool.tile([1, NC], F32, tag="sums", name="sum_ps")
                    ets = {}

                    def at_step(jt):
                        ps = pspool.tile([P, NC], F32, tag="ps", name="ps_at")
                        for rt in range(RT):
                            nc.tensor.matmul(ps[:], KT_t[:, rt, jt * P:(jt + 1) * P],
                                             QT_t[:, rt, isl],
                                             start=(rt == 0), stop=(rt == RT - 1))
                        et = etpool.tile([P, NC], F32R, tag="et", name="et")
                        nc.scalar.activation(et[:], ps[:], AF.Exp, scale=float(SCALE))
                        ets[jt] = et

                    def ev_step(jt):
                        et = ets.pop(jt)
                        for rt in range(RT):
                            nc.tensor.matmul(av_ps[rt][:], V_t[:, jt, rt * P:(rt + 1) * P], et[:],
                                             start=(jt == 0), stop=(jt == JT - 1))
                        nc.tensor.matmul(sum_ps[:], ones[:], et[:],
                                         start=(jt == 0), stop=(jt == JT - 1))

                    for jt in range(JT + LAG):
                        if jt < JT:
                            at_step(jt)
                        if jt >= LAG:
                            ev_step(jt - LAG)

                    sums_sb = smpool.tile([1, NC], F32, tag="sums_sb", name="sums_sb")
                    nc.vector.tensor_copy(sums_sb[:], sum_ps[:])
                    scr = drpool.tile([1, NC], F32, tag="scr", name="scr")
                    nc.scalar.dma_start(scr[:], sums_sb[:])
                    inv = smpool.tile([P, NC // P], F32, tag="inv", name="inv")
                    nc.scalar.dma_start(inv[:], scr[:].rearrange("o (a p) -> p (o a)", p=P))
                    nc.vector.reciprocal(inv[:], inv[:])

                    avt_sb = []
                    for rt in range(RT):
                        t = avpool.tile([P, NC], F32R, tag="avt", name=f"avt_{rt}")
                        nc.vector.tensor_copy(t[:], av_ps[rt][:])
                        avt_sb.append(t)

                    for it in range(NC // P):
                        i0 = ic * NC + it * P
                        ot = opool.tile([P, DIM], F32, tag="out", name="ot")
                        for dc in range(DIM // NC):
                            ps = psopool.tile([P, NC], F32, tag="pso", name="ps_o")
                            for rt in range(RT):
                                nc.tensor.matmul(ps[:], avt_sb[rt][:, it * P:(it + 1) * P],
                                                 wo_t[:, rt, dc * NC:(dc + 1) * NC],
                                                 start=(rt == 0), stop=(rt == RT - 1))
                            nc.vector.tensor_scalar_mul(ot[:, dc * NC:(dc + 1) * NC], ps[:],
                                                        inv[:, it:it + 1])
                        nc.scalar.dma_start(out.ap()[i0:i0 + P, :], ot[:])

            if reps == 1:
                body()
            else:
                with tc.For_i(0, reps, 1) as i:
                    body(i)

    nc.compile()
    return nc


_CACHE = {}


def _get_program():
    if "nc" not in _CACHE:
        _CACHE["nc"] = build_program(reps=1)
    return _CACHE["nc"]


def kernel(q, k, v, Wq, Wk, Wv, Wo):
    nc = _get_program()
    q = np.asarray(q, dtype=np.float32)
    k = np.asarray(k, dtype=np.float32)
    v = np.asarray(v, dtype=np.float32)
    # Zero-FLOP host-side layout prep: transpose so the contraction dim (D)
    # lands on SBUF partitions; one batch per core.
    qT = np.ascontiguousarray(q.transpose(0, 2, 1))
    kT = np.ascontiguousarray(k.transpose(0, 2, 1))
    vT = np.ascontiguousarray(v.transpose(0, 2, 1))
    wqT = np.ascontiguousarray(np.asarray(Wq, dtype=np.float32).T)
    wkT = np.ascontiguousarray(np.asarray(Wk, dtype=np.float32).T)
    wvT = np.ascontiguousarray(np.asarray(Wv, dtype=np.float32).T)
    woT = np.ascontiguousarray(np.asarray(Wo, dtype=np.float32).T)

    in_maps = [{"qT": qT[c], "kT": kT[c], "vT": vT[c],
                "wqT": wqT, "wkT": wkT, "wvT": wvT, "woT": woT}
               for c in range(B)]
    res = bass_utils.run_bass_kernel_spmd(nc, in_maps, core_ids=list(range(B)))
    return np.stack([res.results[c]["out"] for c in range(B)], axis=0)



# revision 2
# speedup vs baseline: 1.5562x; 1.5562x over previous
"""LowRankAttention Trainium2 kernel v2 (Bass/Tile), data-parallel over 8 cores.

Math per batch b (one batch per core):
    Q = q @ Wq^T, K = k @ Wk^T, V = v @ Wv^T          (rank projections, R=256)
    A = softmax(Q K^T / sqrt(R))                       (softmax over keys j)
    out = (A @ V) @ Wo^T

v2 changes vs baseline:
  - q/k/v and weights cast to bf16 on host: input HBM traffic halves
    (27->13.6 MiB per core); all projection matmuls run bf16 (same PE speed).
  - Row sums of E=exp(A) no longer use ones-row matmuls on PE (was 13.6us of
    PE time).  Instead E^T tiles are accumulated over j-tiles on DVE (even jt)
    and GpSimd (odd jt) into two [128,512] accumulators; 8 single-row matmuls
    per chunk then give the per-query sums TRANSPOSED [i,1] directly -- which
    also kills the DRAM round-trip transpose of the old kernel.
  - V projection is interleaved into chunk 0's attention steps (PE no longer
    stalls waiting for v to stream in).
  - Output-projection groups of chunk ic are interleaved into chunk ic+1's
    attention steps; per-query 1/rowsum scaling alternates ACT/DVE.
  - DMA issue order matches consumption: wk, k, wq, q0, wv, v, q1-3, wo.
"""

import numpy as np

import concourse.bacc as bacc
import concourse.mybir as mybir
import concourse.tile as tile
from concourse import bass_utils

F32 = mybir.dt.float32
F32R = mybir.dt.float32r
BF16 = mybir.dt.bfloat16
AF = mybir.ActivationFunctionType
ALU = mybir.AluOpType

DIM, RANK, B, S = 1024, 256, 8, 2048
P = 128
NC = 512                      # moving-operand / psum free chunk
DT = DIM // P                 # 8  d-tiles
RT = RANK // P                # 2  r-tiles
SC = S // NC                  # 4  s-chunks (i-chunks and j-quarters)
JT = S // P                   # 16 j-tiles
IT = NC // P                  # 4  i-tiles per chunk
DC = DIM // NC                # 2  d-chunks of output
SCALE = 1.0 / np.sqrt(np.float32(RANK))
LAG = 2


def build_program(reps: int = 1):
    nc = bacc.Bacc("TRN2", target_bir_lowering=False, debug=False)

    qT = nc.dram_tensor("qT", [DIM, S], BF16, kind="ExternalInput")
    kT = nc.dram_tensor("kT", [DIM, S], BF16, kind="ExternalInput")
    vT = nc.dram_tensor("vT", [DIM, S], BF16, kind="ExternalInput")
    wqT = nc.dram_tensor("wqT", [DIM, RANK], BF16, kind="ExternalInput")
    wkT = nc.dram_tensor("wkT", [DIM, RANK], BF16, kind="ExternalInput")
    wvT = nc.dram_tensor("wvT", [DIM, RANK], BF16, kind="ExternalInput")
    woT = nc.dram_tensor("woT", [RANK, DIM], BF16, kind="ExternalInput")
    out = nc.dram_tensor("out", [S, DIM], F32, kind="ExternalOutput")

    with tile.TileContext(nc) as tc:
        with tc.tile_pool(name="w", bufs=1) as wpool, \
             tc.tile_pool(name="inp", bufs=24) as inpool, \
             tc.tile_pool(name="inq", bufs=12) as qpool, \
             tc.tile_pool(name="per", bufs=1) as perpool, \
             tc.tile_pool(name="qt", bufs=2) as qtpool, \
             tc.tile_pool(name="et", bufs=6) as etpool, \
             tc.tile_pool(name="acc", bufs=4) as accpool, \
             tc.tile_pool(name="av", bufs=4) as avpool, \
             tc.tile_pool(name="o", bufs=3) as opool, \
             tc.tile_pool(name="sm", bufs=2) as smpool, \
             tc.tile_pool(name="ps", bufs=3, space="PSUM") as pspool, \
             tc.tile_pool(name="pso", bufs=2, space="PSUM") as psopool, \
             tc.tile_pool(name="psav", bufs=2, space="PSUM") as psavpool, \
             tc.tile_pool(name="pssum", bufs=1, space="PSUM") as pssumpool:

            def body(_i=None):
                # ---- weight tiles ----
                wk_t = wpool.tile([P, DT, RANK], BF16, tag="wk", name="wk_t")
                wq_t = wpool.tile([P, DT, RANK], BF16, tag="wq", name="wq_t")
                wv_t = wpool.tile([P, DT, RANK], BF16, tag="wv", name="wv_t")
                wo_t = wpool.tile([P, RT, DIM], BF16, tag="wo", name="wo_t")
                ones_f = wpool.tile([P, 1], F32, tag="onesf", name="ones_f")
                nc.vector.memset(ones_f[:], 1.0)
                ones = wpool.tile([P, 1], BF16, tag="ones", name="ones")
                nc.vector.tensor_copy(ones[:], ones_f[:])

                # ---- DMA issue order == consumption order ----
                nc.sync.dma_start(wk_t[:], wkT.ap().rearrange("(dt p) r -> p dt r", p=P))
                ktiles = {}
                for c in range(SC):
                    for dt in range(DT):
                        t = inpool.tile([P, NC], BF16, tag="inKV", name=f"k_{dt}_{c}")
                        nc.sync.dma_start(
                            t[:], kT.ap()[dt * P:(dt + 1) * P, c * NC:(c + 1) * NC])
                        ktiles[(dt, c)] = t
                nc.sync.dma_start(wq_t[:], wqT.ap().rearrange("(dt p) r -> p dt r", p=P))
                qtiles = {}

                def load_q(ic):
                    for dt in range(DT):
                        t = qpool.tile([P, NC], BF16, tag="inQ", name=f"q_{dt}")
                        nc.sync.dma_start(
                            t[:], qT.ap()[dt * P:(dt + 1) * P, ic * NC:(ic + 1) * NC])
                        qtiles[(dt, ic)] = t

                load_q(0)
                nc.sync.dma_start(wv_t[:], wvT.ap().rearrange("(dt p) r -> p dt r", p=P))
                vtiles = {}
                for c in range(SC):
                    for dt in range(DT):
                        t = inpool.tile([P, NC], BF16, tag="inKV", name=f"v_{dt}_{c}")
                        nc.sync.dma_start(
                            t[:], vT.ap()[dt * P:(dt + 1) * P, c * NC:(c + 1) * NC])
                        vtiles[(dt, c)] = t
                for ic in range(1, SC):
                    load_q(ic)
                nc.sync.dma_start(wo_t[:], woT.ap().rearrange("(rt p) d -> p rt d", p=P))

                # ---- persistent projection outputs ----
                KT_t = perpool.tile([P, RT, S], F32R, tag="KT", name="KT_t")   # [r_p, rt, j]
                V_t = perpool.tile([P, JT, RANK], F32R, tag="V", name="V_t")   # [j_p, jt, r]

                # ---- K projection (per j-quarter, straight off the DMA) ----
                for sc in range(SC):
                    for rt in range(RT):
                        ps = pspool.tile([P, NC], F32, tag="ps", name="ps_projk")
                        for dt in range(DT):
                            nc.tensor.matmul(ps[:], wk_t[:, dt, rt * P:(rt + 1) * P],
                                             ktiles[(dt, sc)][:],
                                             start=(dt == 0), stop=(dt == DT - 1))
                        nc.scalar.copy(KT_t[:, rt, sc * NC:(sc + 1) * NC], ps[:])

                def vproj(jt):
                    c, o = jt // IT, (jt % IT) * P
                    ps = psopool.tile([P, RANK], F32, tag="pso", name="ps_v")
                    for dt in range(DT):
                        nc.tensor.matmul(ps[:], vtiles[(dt, c)][:, o:o + P], wv_t[:, dt, :],
                                         start=(dt == 0), stop=(dt == DT - 1))
                    nc.scalar.copy(V_t[:, jt, :], ps[:])

                def qproj(ic):
                    qt = qtpool.tile([P, RT, NC], F32R, tag="qt", name="qt_t")
                    for rt in range(RT):
                        ps = pspool.tile([P, NC], F32, tag="ps", name="ps_projq")
                        for dt in range(DT):
                            nc.tensor.matmul(ps[:], wq_t[:, dt, rt * P:(rt + 1) * P],
                                             qtiles[(dt, ic)][:],
                                             start=(dt == 0), stop=(dt == DT - 1))
                        nc.scalar.copy(qt[:, rt, :], ps[:])
                    return qt

                # out-projection group g of a finished chunk: 2 matmuls + scale
                def outgroup(ctx, g):
                    it, dc = g // DC, g % DC
                    avt, inv, ots, ic = ctx
                    ps = psopool.tile([P, NC], F32, tag="pso", name="ps_o")
                    for rt in range(RT):
                        nc.tensor.matmul(ps[:], avt[rt][:, it * P:(it + 1) * P],
                                         wo_t[:, rt, dc * NC:(dc + 1) * NC],
                                         start=(rt == 0), stop=(rt == RT - 1))
                    if dc == 0:
                        ots[it] = opool.tile([P, DIM], F32, tag="out", name="ot")
                    ot = ots[it]
                    osl = ot[:, dc * NC:(dc + 1) * NC]
                    if g % 2 == 0:
                        nc.scalar.mul(osl, ps[:], inv[:, it:it + 1])
                    else:
                        nc.vector.tensor_scalar_mul(osl, ps[:], inv[:, it:it + 1])
                    if dc == DC - 1:
                        i0 = ic * NC + it * P
                        nc.gpsimd.dma_start(out.ap()[i0:i0 + P, :], ot[:])

                # ---- chunk loop ----
                prev_ctx = None
                for ic in range(SC):
                    qt = qproj(ic)
                    accA = accpool.tile([P, NC], BF16, tag="acc", name="accA")
                    accB = accpool.tile([P, NC], BF16, tag="acc", name="accB")
                    av_ps = [psavpool.tile([P, NC], F32, tag="av", name=f"av_{rt}")
                             for rt in range(RT)]
                    ets = {}

                    def at_step(jt):
                        ps = pspool.tile([P, NC], F32, tag="ps", name="ps_at")
                        for rt in range(RT):
                            nc.tensor.matmul(ps[:], KT_t[:, rt, jt * P:(jt + 1) * P],
                                             qt[:, rt, :],
                                             start=(rt == 0), stop=(rt == RT - 1))
                        et = etpool.tile([P, NC], F32R, tag="et", name="et")
                        nc.scalar.activation(et[:], ps[:], AF.Exp, scale=float(SCALE))
                        ets[jt] = et
                        # accumulate E^T tiles for the row sums (partition dim
                        # reduction happens later via 1-row matmuls)
                        eng, acc = (nc.vector, accA) if jt % 2 == 0 else (nc.gpsimd, accB)
                        if jt < 2:
                            eng.tensor_copy(acc[:], et[:])
                        else:
                            eng.tensor_tensor(acc[:], acc[:], et[:], op=ALU.add)

                    def ev_step(jt):
                        et = ets.pop(jt)
                        for rt in range(RT):
                            nc.tensor.matmul(av_ps[rt][:], V_t[:, jt, rt * P:(rt + 1) * P],
                                             et[:],
                                             start=(jt == 0), stop=(jt == JT - 1))

                    for jt in range(JT):
                        at_step(jt)
                        if ic == 0:
                            vproj(jt)
                        elif jt < DC * IT:
                            outgroup(prev_ctx, jt)
                        if jt >= LAG:
                            ev_step(jt - LAG)
                    for jt in range(JT - LAG, JT):
                        ev_step(jt)

                    # transposed row sums: 1-row matmuls off the accumulators
                    sums_ps = pssumpool.tile([P, IT], F32, tag="sums", name="sums_ps")
                    for b in range(IT):
                        nc.tensor.matmul(sums_ps[:, b:b + 1],
                                         accA[:, b * P:(b + 1) * P], ones[:],
                                         start=True, stop=False)
                        nc.tensor.matmul(sums_ps[:, b:b + 1],
                                         accB[:, b * P:(b + 1) * P], ones[:],
                                         start=False, stop=True)

                    avt = []
                    for rt in range(RT):
                        t = avpool.tile([P, NC], BF16, tag="avt", name=f"avt_{rt}")
                        if rt == 0:
                            nc.vector.tensor_copy(t[:], av_ps[rt][:])
                        else:
                            nc.scalar.copy(t[:], av_ps[rt][:])
                        avt.append(t)
                    inv = smpool.tile([P, IT], F32, tag="inv", name="inv")
                    nc.vector.reciprocal(inv[:], sums_ps[:])

                    prev_ctx = (avt, inv, {}, ic)

                # drain the last chunk's output projection
                for g in range(DC * IT):
                    outgroup(prev_ctx, g)

            if reps == 1:
                body()
            else:
                with tc.For_i(0, reps, 1) as i:
                    body(i)

    nc.compile()
    return nc


_CACHE = {}


def _get_program():
    if "nc" not in _CACHE:
        _CACHE["nc"] = build_program(reps=1)
    return _CACHE["nc"]


def prep_inputs(q, k, v, Wq, Wk, Wv, Wo):
    """Host-side layout/dtype prep: transpose so the contraction dim (D) lands
    on SBUF partitions, cast to bf16; one batch per core."""
    import ml_dtypes
    bf16 = ml_dtypes.bfloat16
    qT = np.ascontiguousarray(np.asarray(q, np.float32).transpose(0, 2, 1)).astype(bf16)
    kT = np.ascontiguousarray(np.asarray(k, np.float32).transpose(0, 2, 1)).astype(bf16)
    vT = np.ascontiguousarray(np.asarray(v, np.float32).transpose(0, 2, 1)).astype(bf16)
    wqT = np.ascontiguousarray(np.asarray(Wq, np.float32).T).astype(bf16)
    wkT = np.ascontiguousarray(np.asarray(Wk, np.float32).T).astype(bf16)
    wvT = np.ascontiguousarray(np.asarray(Wv, np.float32).T).astype(bf16)
    woT = np.ascontiguousarray(np.asarray(Wo, np.float32).T).astype(bf16)
    return [{"qT": qT[c], "kT": kT[c], "vT": vT[c],
             "wqT": wqT, "wkT": wkT, "wvT": wvT, "woT": woT}
            for c in range(B)]


def kernel(q, k, v, Wq, Wk, Wv, Wo):
    nc = _get_program()
    in_maps = prep_inputs(q, k, v, Wq, Wk, Wv, Wo)
    res = bass_utils.run_bass_kernel_spmd(nc, in_maps, core_ids=list(range(B)))
    return np.stack([res.results[c]["out"] for c in range(B)], axis=0)


# revision 3
# speedup vs baseline: 2.0746x; 1.3331x over previous
"""LowRankAttention Trainium2 kernel v2 (Bass/Tile), data-parallel over 8 cores.

Math per batch b (one batch per core):
    Q = q @ Wq^T, K = k @ Wk^T, V = v @ Wv^T          (rank projections, R=256)
    A = softmax(Q K^T / sqrt(R))                       (softmax over keys j)
    out = (A @ V) @ Wo^T

v2 changes vs baseline:
  - q/k/v and weights cast to bf16 on host: input HBM traffic halves
    (27->13.6 MiB per core); all projection matmuls run bf16 (same PE speed).
  - Row sums of E=exp(A) no longer use ones-row matmuls on PE (was 13.6us of
    PE time).  Instead E^T tiles are accumulated over j-tiles on DVE (even jt)
    and GpSimd (odd jt) into two [128,512] accumulators; 8 single-row matmuls
    per chunk then give the per-query sums TRANSPOSED [i,1] directly -- which
    also kills the DRAM round-trip transpose of the old kernel.
  - V projection is interleaved into chunk 0's attention steps (PE no longer
    stalls waiting for v to stream in).
  - Output-projection groups of chunk ic are interleaved into chunk ic+1's
    attention steps; per-query 1/rowsum scaling alternates ACT/DVE.
  - DMA issue order matches consumption: wk, k, wq, q0, wv, v, q1-3, wo.
"""

import numpy as np

import concourse.bacc as bacc
import concourse.mybir as mybir
import concourse.tile as tile
from concourse import bass_utils

F32 = mybir.dt.float32
F32R = mybir.dt.float32r
BF16 = mybir.dt.bfloat16
AF = mybir.ActivationFunctionType
ALU = mybir.AluOpType

DIM, RANK, B, S = 1024, 256, 8, 2048
P = 128
NC = 512                      # moving-operand / psum free chunk
DT = DIM // P                 # 8  d-tiles
RT = RANK // P                # 2  r-tiles
SC = S // NC                  # 4  s-chunks (i-chunks and j-quarters)
JT = S // P                   # 16 j-tiles
IT = NC // P                  # 4  i-tiles per chunk
DC = DIM // NC                # 2  d-chunks of output
SCALE = 1.0 / np.sqrt(np.float32(RANK))
LAG = 2


def build_program(reps: int = 1, unroll: int = 1):
    nc = bacc.Bacc("TRN2", target_bir_lowering=False, debug=False)

    qT = nc.dram_tensor("qT", [DIM, S], BF16, kind="ExternalInput")
    kT = nc.dram_tensor("kT", [DIM, S], BF16, kind="ExternalInput")
    vT = nc.dram_tensor("vT", [DIM, S], BF16, kind="ExternalInput")
    wqT = nc.dram_tensor("wqT", [DIM, RANK], BF16, kind="ExternalInput")
    wkT = nc.dram_tensor("wkT", [DIM, RANK], BF16, kind="ExternalInput")
    wvT = nc.dram_tensor("wvT", [DIM, RANK], BF16, kind="ExternalInput")
    woT = nc.dram_tensor("woT", [RANK, DIM], BF16, kind="ExternalInput")
    out = nc.dram_tensor("out", [S, DIM], F32, kind="ExternalOutput")

    with tile.TileContext(nc) as tc:
        with tc.tile_pool(name="w", bufs=1) as wpool, \
             tc.tile_pool(name="inp", bufs=20) as inpool, \
             tc.tile_pool(name="inq", bufs=4) as qpool, \
             tc.tile_pool(name="per", bufs=1) as perpool, \
             tc.tile_pool(name="qt", bufs=2) as qtpool, \
             tc.tile_pool(name="et", bufs=6) as etpool, \
             tc.tile_pool(name="acc", bufs=4) as accpool, \
             tc.tile_pool(name="av", bufs=4) as avpool, \
             tc.tile_pool(name="o", bufs=3) as opool, \
             tc.tile_pool(name="sm", bufs=2) as smpool, \
             tc.tile_pool(name="ps", bufs=3, space="PSUM") as pspool, \
             tc.tile_pool(name="pso", bufs=3, space="PSUM") as psopool, \
             tc.tile_pool(name="psav", bufs=2, space="PSUM") as psavpool:

            def body(_i=None):
                # ---- weight tiles ----
                wk_t = wpool.tile([P, DT, RANK], BF16, tag="wk", name="wk_t")
                wq_t = wpool.tile([P, DT, RANK], BF16, tag="wq", name="wq_t")
                wv_t = wpool.tile([P, DT, RANK], BF16, tag="wv", name="wv_t")
                wo_t = wpool.tile([P, RT, DIM], BF16, tag="wo", name="wo_t")
                ones_f = wpool.tile([P, 1], F32, tag="onesf", name="ones_f")
                nc.vector.memset(ones_f[:], 1.0)
                ones = wpool.tile([P, 1], BF16, tag="ones", name="ones")
                nc.vector.tensor_copy(ones[:], ones_f[:])

                # ---- DMA issue order == consumption order; few, large DMAs
                # (each DMA costs ~0.6us of HWDGE config time regardless of
                # size, so granularity is halves/whole-tensors, not quarters)
                H = S // 2
                nc.sync.dma_start(wk_t[:], wkT.ap().rearrange("(dt p) r -> p dt r", p=P))
                ktiles = {}
                for h in range(2):
                    for dt in range(DT):
                        t = inpool.tile([P, H], BF16, tag="inKV", name=f"k_{dt}_{h}")
                        nc.sync.dma_start(
                            t[:], kT.ap()[dt * P:(dt + 1) * P, h * H:(h + 1) * H])
                        ktiles[(dt, h)] = t
                nc.sync.dma_start(wq_t[:], wqT.ap().rearrange("(dt p) r -> p dt r", p=P))
                qtiles = {}

                def load_q(ic):
                    t = qpool.tile([P, DT, NC], BF16, tag="inQ", name=f"q_{ic}")
                    nc.sync.dma_start(
                        t[:], qT.ap()[:, ic * NC:(ic + 1) * NC]
                        .rearrange("(dt p) c -> p dt c", p=P))
                    qtiles[ic] = t

                load_q(0)
                nc.sync.dma_start(wv_t[:], wvT.ap().rearrange("(dt p) r -> p dt r", p=P))
                vtiles = {}
                for h in range(2):
                    for dt in range(DT):
                        t = inpool.tile([P, H], BF16, tag="inKV", name=f"v_{dt}_{h}")
                        nc.sync.dma_start(
                            t[:], vT.ap()[dt * P:(dt + 1) * P, h * H:(h + 1) * H])
                        vtiles[(dt, h)] = t
                for ic in range(1, SC):
                    load_q(ic)
                nc.sync.dma_start(wo_t[:], woT.ap().rearrange("(rt p) d -> p rt d", p=P))

                # ---- persistent projection outputs ----
                KT_t = perpool.tile([P, RT, S], F32R, tag="KT", name="KT_t")   # [r_p, rt, j]
                V_t = perpool.tile([P, JT, RANK], F32R, tag="V", name="V_t")   # [j_p, jt, r]

                # ---- K projection (per s-chunk, straight off the DMA) ----
                for sc in range(SC):
                    h, o = sc // 2, (sc % 2) * NC
                    for rt in range(RT):
                        ps = pspool.tile([P, NC], F32, tag="ps", name="ps_projk")
                        for dt in range(DT):
                            nc.tensor.matmul(ps[:], wk_t[:, dt, rt * P:(rt + 1) * P],
                                             ktiles[(dt, h)][:, o:o + NC],
                                             start=(dt == 0), stop=(dt == DT - 1))
                        nc.scalar.copy(KT_t[:, rt, sc * NC:(sc + 1) * NC], ps[:])

                def vproj(jt):
                    h, o = jt // 8, (jt % 8) * P
                    ps = psopool.tile([P, RANK], F32, tag="pso", name="ps_v")
                    for dt in range(DT):
                        nc.tensor.matmul(ps[:], vtiles[(dt, h)][:, o:o + P], wv_t[:, dt, :],
                                         start=(dt == 0), stop=(dt == DT - 1))
                    nc.scalar.copy(V_t[:, jt, :], ps[:])

                def qproj(ic):
                    qt = qtpool.tile([P, RT, NC], F32R, tag="qt", name="qt_t")
                    for rt in range(RT):
                        ps = pspool.tile([P, NC], F32, tag="ps", name="ps_projq")
                        for dt in range(DT):
                            nc.tensor.matmul(ps[:], wq_t[:, dt, rt * P:(rt + 1) * P],
                                             qtiles[ic][:, dt, :],
                                             start=(dt == 0), stop=(dt == DT - 1))
                        nc.scalar.copy(qt[:, rt, :], ps[:])
                    return qt

                # out-projection group g of a finished chunk: 2 matmuls + scale
                def outgroup(ctx, g):
                    it, dc = g // DC, g % DC
                    avt, inv, ic = ctx
                    ps = psopool.tile([P, NC], F32, tag="pso", name="ps_o")
                    for rt in range(RT):
                        nc.tensor.matmul(ps[:], avt[rt][:, it * P:(it + 1) * P],
                                         wo_t[:, rt, dc * NC:(dc + 1) * NC],
                                         start=(rt == 0), stop=(rt == RT - 1))
                    ot = opool.tile([P, NC], F32, tag="out", name="ot")
                    if g % 2 == 0:
                        nc.scalar.mul(ot[:], ps[:], inv[:, it:it + 1])
                    else:
                        nc.vector.tensor_scalar_mul(ot[:], ps[:], inv[:, it:it + 1])
                    i0 = ic * NC + it * P
                    nc.sync.dma_start(out.ap()[i0:i0 + P, dc * NC:(dc + 1) * NC], ot[:])

                # ---- chunk loop ----
                prev_ctx = None
                qt = qproj(0)
                for ic in range(SC):
                    accA = accpool.tile([P, NC], BF16, tag="acc", name="accA")
                    accB = accpool.tile([P, NC], BF16, tag="acc", name="accB")
                    av_ps = [psavpool.tile([P, NC], F32, tag="av", name=f"av_{rt}")
                             for rt in range(RT)]
                    ets = {}

                    def at_step(jt, qt=qt, accA=accA, accB=accB, ets=ets):
                        ps = pspool.tile([P, NC], F32, tag="ps", name="ps_at")
                        for rt in range(RT):
                            nc.tensor.matmul(ps[:], KT_t[:, rt, jt * P:(jt + 1) * P],
                                             qt[:, rt, :],
                                             start=(rt == 0), stop=(rt == RT - 1))
                        et = etpool.tile([P, NC], F32R, tag="et", name="et")
                        nc.scalar.activation(et[:], ps[:], AF.Exp, scale=float(SCALE))
                        ets[jt] = et
                        # accumulate E^T tiles for the row sums (partition dim
                        # reduction happens later via 1-row matmuls); odd jt on
                        # DVE so the last tile's add is on the faster engine
                        eng, acc = (nc.vector, accA) if jt % 2 == 1 else (nc.gpsimd, accB)
                        if jt < 2:
                            eng.tensor_copy(acc[:], et[:])
                        else:
                            eng.tensor_tensor(acc[:], acc[:], et[:], op=ALU.add)

                    def ev_step(jt, av_ps=av_ps, ets=ets):
                        et = ets.pop(jt)
                        for rt in range(RT):
                            nc.tensor.matmul(av_ps[rt][:], V_t[:, jt, rt * P:(rt + 1) * P],
                                             et[:],
                                             start=(jt == 0), stop=(jt == JT - 1))

                    for jt in range(JT):
                        at_step(jt)
                        if ic == 0:
                            vproj(jt)
                        elif jt < DC * IT:
                            outgroup(prev_ctx, jt)
                        if jt >= LAG:
                            ev_step(jt - LAG)
                    for jt in range(JT - LAG, JT):
                        ev_step(jt)

                    # avt copies first (DVE + ACT), then next chunk's Q
                    # projection fills the PE while the accumulator adds and
                    # copies land, then the tiny transposed row-sum matmuls
                    avt = []
                    for rt in range(RT):
                        t = avpool.tile([P, NC], BF16, tag="avt", name=f"avt_{rt}")
                        if rt == 0:
                            nc.vector.tensor_copy(t[:], av_ps[rt][:])
                        else:
                            nc.scalar.copy(t[:], av_ps[rt][:])
                        avt.append(t)
                    if ic + 1 < SC:
                        qt = qproj(ic + 1)

                    sums_ps = psopool.tile([P, IT], F32, tag="pso", name="sums_ps")
                    for b in range(IT):
                        nc.tensor.matmul(sums_ps[:, b:b + 1],
                                         accA[:, b * P:(b + 1) * P], ones[:],
                                         start=True, stop=False)
                        nc.tensor.matmul(sums_ps[:, b:b + 1],
                                         accB[:, b * P:(b + 1) * P], ones[:],
                                         start=False, stop=True)
                    inv = smpool.tile([P, IT], F32, tag="inv", name="inv")
                    nc.vector.reciprocal(inv[:], sums_ps[:])

                    prev_ctx = (avt, inv, {}, ic)

                # drain the last chunk's output projection
                for g in range(DC * IT):
                    outgroup(prev_ctx, g)

            if reps == 1:
                for _ in range(unroll):
                    body()
            else:
                with tc.For_i(0, reps, 1) as i:
                    body(i)

    nc.compile()
    return nc


_CACHE = {}


def _get_program():
    if "nc" not in _CACHE:
        _CACHE["nc"] = build_program(reps=1)
    return _CACHE["nc"]


def prep_inputs(q, k, v, Wq, Wk, Wv, Wo):
    """Host-side layout/dtype prep: transpose so the contraction dim (D) lands
    on SBUF partitions, cast to bf16; one batch per core."""
    import ml_dtypes
    bf16 = ml_dtypes.bfloat16
    qT = np.ascontiguousarray(np.asarray(q, np.float32).transpose(0, 2, 1)).astype(bf16)
    kT = np.ascontiguousarray(np.asarray(k, np.float32).transpose(0, 2, 1)).astype(bf16)
    vT = np.ascontiguousarray(np.asarray(v, np.float32).transpose(0, 2, 1)).astype(bf16)
    wqT = np.ascontiguousarray(np.asarray(Wq, np.float32).T).astype(bf16)
    wkT = np.ascontiguousarray(np.asarray(Wk, np.float32).T).astype(bf16)
    wvT = np.ascontiguousarray(np.asarray(Wv, np.float32).T).astype(bf16)
    woT = np.ascontiguousarray(np.asarray(Wo, np.float32).T).astype(bf16)
    return [{"qT": qT[c], "kT": kT[c], "vT": vT[c],
             "wqT": wqT, "wkT": wkT, "wvT": wvT, "woT": woT}
            for c in range(B)]


def kernel(q, k, v, Wq, Wk, Wv, Wo):
    nc = _get_program()
    in_maps = prep_inputs(q, k, v, Wq, Wk, Wv, Wo)
    res = bass_utils.run_bass_kernel_spmd(nc, in_maps, core_ids=list(range(B)))
    return np.stack([res.results[c]["out"] for c in range(B)], axis=0)
